# revision 26
# baseline (speedup 1.0000x reference)
"""Trainium2 Bass kernel for nn_Attention_48687749267827.

Dense transformer attention block (1x1-conv QKV + windowed relative-position
bias + softmax + 1x1-conv out-proj + layer-scale), data-parallel over batch
across 8 NeuronCores (2 batches per core).

Design notes (per core):
  * Attention is computed in transposed orientation: S^T[m, n] tiles of
    [112, 784] so that the AV product needs no on-chip transposes.  The
    softmax denominator falls out of ones-columns appended to V^T.
  * The relative-position bias is Toeplitz: B[m, n] = db[h, nt - mt + C]
    (nt = 55*rn + cn).  A DRAM band table T3[h, p, u] = db[h, u - s(p)]
    (s(p) = (p%28) + 55*(p//28)) makes the per-(h, mt) shifted replica a
    SINGLE strided DMA: partition p reads T3[h, p, base(mt) : +1513] with
    base(mt) = (27-4mt)*55 + 27, i.e. source offset affine in p.
  * The bias is added on the TensorEngine with an identity matmul
    (start=False PSUM accumulation) reading an [rn, cn]-window view of the
    replica - no gather, no extra vector work.
  * softmax skips the max-subtraction (logits are O(1)); P = exp(S^T + B^T)
    on ScalarE, written bf16.  AV matmuls run 2 tiles BEHIND the S/exp
    stream (software pipelining) so the PE never waits on the ScalarE:
    keeps the PE instruction stream dense and the HAM clock-gate at 8/8.
  * PSUM budget exactly 8 banks: s-tiles [112,2,512] x bufs=2 (4 banks) +
    av accumulators [128,2,512] x 2 batches x bufs=1 (4 banks).
  * Per-head-pair normalize: denominator rows are DMA-gathered once,
    one DVE reciprocal, and the 4 normalize multiplies run on the (idle)
    GpSimd engine, all off the PE critical path.
  * All matmuls are bf16 with fp32 PSUM accumulation.
"""

import os
import sys

for _p in ("/opt/trn_rl_repo", "/root/.axon_site/_ro/trn_rl_repo"):
    if os.path.isdir(_p) and _p not in sys.path:
        sys.path.insert(0, _p)

from contextlib import ExitStack

import numpy as np

import concourse.bass as bass
import concourse.tile as tile
import concourse.mybir as mybir
from concourse import bacc
from concourse.bass import ds, ts
from concourse.masks import make_identity

# ---------------------------------------------------------------- constants
B, C_IN, H, W = 16, 384, 28, 28
NUM_HEADS, HEAD_DIM = 12, 32
MID = NUM_HEADS * HEAD_DIM  # 384
OUT = 384
SCALE = HEAD_DIM ** -0.5
N = H * W                   # 784
NCORES = 8
BPC = B // NCORES           # 2 batches per core
DD = 2 * H - 1              # 55
NBIAS = DD * DD             # 3025
MT = 112                    # m-tile rows (4 rm-rows x 28 cm)
NMT = N // MT               # 7
REPW = (H - 1) * DD + (W - 1) + 1   # 1513 window length per partition
NC0 = 392                   # n-chunk (14*28, bank-aligned via padded psum)
EXTW = NBIAS + W            # 3053 band-table width (cm shifts)
T3W = NBIAS                 # 3025 T3 inner width

F32 = mybir.dt.float32
BF16 = mybir.dt.bfloat16
FP8 = mybir.dt.float8e4

AOP = mybir.AluOpType
AFT = mybir.ActivationFunctionType


def _build_program():
    nc = bacc.Bacc("TRN2", target_bir_lowering=False, debug=False)

    # ------------------------------------------------ DRAM I/O declarations
    x_d = nc.dram_tensor("x", [BPC, C_IN, N], F32, kind="ExternalInput")
    wqT_d = nc.dram_tensor("wqT", [C_IN, MID], F32, kind="ExternalInput")
    wkT_d = nc.dram_tensor("wkT", [C_IN, MID], F32, kind="ExternalInput")
    wvT_d = nc.dram_tensor("wvT", [C_IN, MID], F32, kind="ExternalInput")
    wpT0_d = nc.dram_tensor("wpT0", [768, OUT], F32, kind="ExternalInput")
    wpT1_d = nc.dram_tensor("wpT1", [768, OUT], F32, kind="ExternalInput")
    bq_d = nc.dram_tensor("bq", [MID], F32, kind="ExternalInput")
    bk_d = nc.dram_tensor("bk", [MID], F32, kind="ExternalInput")
    bp_d = nc.dram_tensor("bp", [OUT], F32, kind="ExternalInput")
    gm_d = nc.dram_tensor("gm", [OUT], F32, kind="ExternalInput")
    db_d = nc.dram_tensor("db", [NUM_HEADS, NBIAS], F32, kind="ExternalInput")
    out_d = nc.dram_tensor("out", [BPC, OUT, N], F32, kind="ExternalOutput")

    with ExitStack() as ctx:
        tc = ctx.enter_context(tile.TileContext(nc))
        const = ctx.enter_context(tc.tile_pool(name="const", bufs=1))
        dram = ctx.enter_context(tc.tile_pool(name="dram", bufs=1, space="DRAM"))
        stage_ctx = ExitStack()
        stage = stage_ctx.enter_context(tc.tile_pool(name="stage", bufs=2))

        # ---------------------------------------- phase 0: weights & tables
        def load_cast(dsrc, shape3, tag):
            w = stage.tile(shape3, F32, tag="wstage")
            nc.sync.dma_start(w[:], dsrc[:].rearrange("(a p) m -> p a m", p=128))
            o = const.tile(shape3, BF16, tag=tag)
            nc.vector.tensor_copy(o[:], w[:])
            return o

        wqT = load_cast(wqT_d, [128, 3, MID], "wqT")
        wkT = load_cast(wkT_d, [128, 3, MID], "wkT")
        wvT = load_cast(wvT_d, [128, 3, MID], "wvT")
        wpT = [load_cast(wpT0_d, [128, 6, OUT], "wpT0"),
               load_cast(wpT1_d, [128, 6, OUT], "wpT1")]

        def load_vec(dsrc, cols, tag):
            o = const.tile([128, cols], F32, tag=tag)
            nc.sync.dma_start(o[:], dsrc[:].rearrange("(a p) -> p a", p=128))
            return o

        bq_sb = load_vec(bq_d, 3, "bq")
        bk_sb = load_vec(bk_d, 3, "bk")
        bp_sb = load_vec(bp_d, 3, "bp")
        gm_sb = load_vec(gm_d, 3, "gm")

        # bias table -> fp8 -> DRAM band table T3[h, p, u] = db[h, u - s(p)]
        # with s(p) = (p % 28) + 55 * (p // 28); built via the cm-shift table
        # db_ext[h, cm, t] = db[h, t - cm] then 4 row-block copies per a.
        # Issued after the x/weight loads so phase 1 starts promptly.
        dbf = stage.tile([NUM_HEADS, NBIAS], F32, tag="dbstage")
        nc.sync.dma_start(dbf[:], db_d[:])
        dbb = const.tile([NUM_HEADS, NBIAS], FP8, tag="dbb")
        nc.vector.tensor_copy(dbb[:], dbf[:])
        db_bf = dram.tile([NUM_HEADS, NBIAS], FP8, tag="db_bf")
        nc.sync.dma_start(db_bf[:], dbb[:])
        db_ext = dram.tile([NUM_HEADS, W, EXTW], FP8, tag="db_ext")
        for cm in range(W):
            nc.sync.dma_start(db_ext[:, cm, cm:cm + NBIAS], db_bf[:])
        t3 = dram.tile([NUM_HEADS, 4, W, T3W], FP8, tag="t3")
        for a in range(4):
            nc.sync.dma_start(t3[:, a, :, 55 * a:NBIAS],
                              db_ext[:, :, 0:NBIAS - 55 * a])
        t3_ap = t3[:]

        identb = const.tile([MT, MT], BF16, tag="identb")
        make_identity(nc, identb[:])
        ident = const.tile([MT, MT], FP8, tag="ident")
        nc.vector.tensor_copy(ident[:], identb[:])

        # HAM warm-up: ~6us of dense dummy matmuls at program start flips the
        # PE clock gate to 8/8 while the input DMAs are still in flight.
        warm = const.tile([128, 512], BF16, tag="warm")
        warmout = const.tile([128, 512], F32, tag="warmout")
        nc.vector.memset(warm[:], 0.0)

        # ---------------------------------------- per-batch persistent sbuf
        xf = [const.tile([128, 3, N], BF16, tag=f"xf{b}", name=f"xf{b}") for b in range(BPC)]
        q_sb = [const.tile([128, 3, N], BF16, tag=f"q{b}", name=f"q{b}") for b in range(BPC)]
        k_sb = [const.tile([128, 3, N], BF16, tag=f"k{b}", name=f"k{b}") for b in range(BPC)]
        vT = [const.tile([MT, NMT, NUM_HEADS, 2 * HEAD_DIM], BF16, tag=f"vT{b}",
                          name=f"vT{b}") for b in range(BPC)]
        omid = [const.tile([128, 6, N], BF16, tag=f"om{b}", name=f"om{b}") for b in range(BPC)]

        for b in range(BPC):
            xs = stage.tile([128, 3, N], F32, tag="xstage", bufs=2)
            nc.sync.dma_start(xs[:], x_d[b].rearrange("(a p) n -> p a n", p=128))
            nc.vector.tensor_copy(xf[b][:], xs[:])
            nc.gpsimd.memset(omid[b][:], 0.0)
            if b == 0:
                nc.vector.memset(vT[b][:, :, :, HEAD_DIM:], 1.0)
            else:
                nc.vector.memset(vT[b][:, :, :, :HEAD_DIM], 1.0)

        NCHUNKS = ((0, 512), (512, N - 512))

        # ------------------------------------------- phase 1: q, k, v^T
        with tc.tile_pool(name="pp1", bufs=2, space="PSUM") as pp1:
            wpst = pp1.tile([128, 2, 512], F32, tag="ps", bufs=3, name="warmps")
            wps = wpst[:, 0, :]
            for wi in range(16):
                nc.tensor.matmul(wps, lhsT=warm[:, :128], rhs=warm[:],
                                 start=True, stop=True)
            nc.vector.tensor_copy(warmout[:], wps)
            for b in range(BPC):
                for mo in range(3):
                    ps = pp1.tile([128, 2, 512], F32, tag="ps", bufs=3)
                    for kc in range(3):
                        for c, (n0, nn) in enumerate(NCHUNKS):
                            nc.tensor.matmul(
                                ps[:, c, :nn],
                                lhsT=wqT[:, kc, ts(mo, 128)],
                                rhs=xf[b][:, kc, n0:n0 + nn],
                                start=(kc == 0), stop=(kc == 2))
                    for c, (n0, nn) in enumerate(NCHUNKS):
                        nc.vector.tensor_scalar(
                            q_sb[b][:, mo, n0:n0 + nn], ps[:, c, :nn],
                            bq_sb[:, mo:mo + 1], SCALE, AOP.add, AOP.mult)
                for mo in range(3):
                    ps = pp1.tile([128, 2, 512], F32, tag="ps", bufs=3)
                    for kc in range(3):
                        for c, (n0, nn) in enumerate(NCHUNKS):
                            nc.tensor.matmul(
                                ps[:, c, :nn],
                                lhsT=wkT[:, kc, ts(mo, 128)],
                                rhs=xf[b][:, kc, n0:n0 + nn],
                                start=(kc == 0), stop=(kc == 2))
                    for c, (n0, nn) in enumerate(NCHUNKS):
                        nc.vector.tensor_scalar(
                            k_sb[b][:, mo, n0:n0 + nn], ps[:, c, :nn],
                            bk_sb[:, mo:mo + 1], None, AOP.add)
                for nt in range(NMT):
                    ps2 = pp1.tile([MT, MID], F32, tag="ps2", bufs=2)
                    for kc in range(3):
                        nc.tensor.matmul(
                            ps2[:],
                            lhsT=xf[b][:, kc, ts(nt, MT)],
                            rhs=wvT[:, kc, :],
                            start=(kc == 0), stop=(kc == 2))
                    vdst = (vT[b][:, nt, :, :HEAD_DIM] if b == 0
                            else vT[b][:, nt, :, HEAD_DIM:])
                    nc.vector.tensor_copy(
                        vdst,
                        ps2[:].rearrange("p (h d) -> p h d", h=NUM_HEADS))

        stage_ctx.close()       # release staging SBUF before phase 2 pools

        # ------------------------------------------- phase 2: attention
        # S matmuls use the FULL 128-partition k tile as lhsT against a
        # zero-masked per-head q copy (q4): rows outside head h's 32-row band
        # are zero, so cross-head terms vanish and the result is exact.  This
        # raises PE array occupancy for S from 22% to 87.5%, which keeps the
        # HAM activity monitor above its un-throttle threshold (the PE clock
        # gate grants 8/8 only to high-occupancy streams).
        # Batch-major: one batch per (b, t) pass so the AV accumulator is a
        # single 2-bank PSUM tile and the s-tiles triple-buffer (6 banks).
        # The 3-deep s-buffer decouples the PE from the ScalarE exp latency:
        # S(i+3) only waits on exp(i), which is long done - the PE stream
        # stays dense and the HAM clock-gate holds 8/8.  AV matmuls run 2
        # tiles behind the S/exp stream; fp8 bias replicas are re-fetched per
        # batch pass (2 x 14.2 MB total, single strided DMA per tile).
        PF = 4           # rep prefetch depth, (b, t, mt) iterations
        AVLAG = 4        # AV matmuls trail the S/exp stream by this many tiles
        with tc.tile_pool(name="spool", bufs=3, space="PSUM") as spool, \
             tc.tile_pool(name="avpool", bufs=1, space="PSUM") as avpool, \
             tc.tile_pool(name="q4p", bufs=1) as q4pool, \
             tc.tile_pool(name="rep", bufs=2 * (PF + 1)) as reppool, \
             tc.tile_pool(name="pt", bufs=8) as ptpool, \
             tc.tile_pool(name="drp", bufs=2) as drpool:

            q4 = q4pool.tile([128, NUM_HEADS, N], BF16, tag="q4")
            nc.vector.memset(q4[:], 0.0)

            def build_q4(b):
                # q4[:, h, :] = head h's 32-row band of q, other rows zero;
                # bands are head-position invariant so zeros persist across
                # batch passes.
                for h in range(NUM_HEADS):
                    hb, hc = 32 * (h % 4), h // 4
                    nc.vector.tensor_copy(
                        q4[ds(hb, 32), h, :], q_sb[b][ds(hb, 32), hc, :])

            iters = [(b, t, mt)
                     for b in range(BPC) for t in range(6) for mt in range(NMT)]
            reps_q = {}

            def prefetch(it_idx):
                if it_idx >= len(iters):
                    return
                b, t, mt = iters[it_idx]
                base = (27 - 4 * mt) * 55 + 27
                pair = []
                for j in range(2):
                    h = 2 * t + j
                    rp = reppool.tile([MT, H, DD], FP8, tag="rep")
                    rpf = rp[:].rearrange("p a b -> p (a b)")
                    src = bass.AP(
                        tensor=t3_ap.tensor,
                        offset=t3_ap.offset + h * (4 * W * T3W) + base,
                        ap=[[T3W, MT], [1, REPW]])
                    nc.sync.dma_start(rpf[:, :REPW], src)
                    pair.append(rp)
                reps_q[(b, t, mt)] = pair

            for i in range(PF):
                prefetch(i)

            pend = []          # (b, t, j, mt, pt) awaiting AV emission
            av_cur = {}        # (b, t) -> avt psum tile, created at first AV

            def emit_av(b, t, j, mt, pt):
                if (b, t) not in av_cur:
                    av_cur[(b, t)] = avpool.tile(
                        [128, 2, 512], F32, tag="av", name=f"av{b}_{t}")
                avt = av_cur[(b, t)]
                h = 2 * t + j
                for c in range(2):
                    n0 = c * NC0
                    nc.tensor.matmul(
                        avt[ds(64 * j, 64), c, :NC0],
                        lhsT=vT[b][:, mt, h, :],
                        rhs=pt[:, n0:n0 + NC0],
                        start=(mt == 0), stop=(mt == NMT - 1),
                        skip_group_check=True)
                if mt == NMT - 1 and j == 1:
                    normalize(b, t, av_cur.pop((b, t)))

            def normalize(b, t, avt):
                # omid rows = av[V rows] / D (D accumulated in the ones rows);
                # denominator row-gather on GpSimd's SWDGE ring so the sync
                # ring stays dedicated to rep prefetch.
                avsb = drpool.tile([128, N], F32, tag="avs", name=f"avs{b}_{t}")
                nc.vector.tensor_copy(
                    avsb[:].rearrange("p (c n) -> p c n", c=2),
                    avt[:, :, :NC0])
                drec = drpool.tile([128, N], F32, tag="drec")
                drecR = drpool.tile([128, N], F32, tag="drecR")
                for j in range(2):
                    srcrow = 64 * j + (32 if b == 0 else 0)
                    dstrow = 64 * j + 32 * b
                    nc.gpsimd.dma_start(
                        drec[ds(dstrow, 32), :],
                        avsb[ds(srcrow, 32), :])
                nc.vector.reciprocal_approx_fast(drecR[:], drec[:])
                for j in range(2):
                    orow = 64 * j + 32 * b
                    nc.gpsimd.tensor_tensor(
                        omid[b][ds(orow, 32), t, :],
                        avsb[ds(orow, 32), :],
                        drecR[ds(orow, 32), :],
                        AOP.mult)

            last_b = None
            for it, (b, t, mt) in enumerate(iters):
                if b != last_b:
                    build_q4(b)
                    last_b = b
                prefetch(it + PF)
                reps = reps_q.pop((b, t, mt))
                for j in range(2):
                    h = 2 * t + j
                    hc = h // 4
                    s_t = spool.tile([MT, 2, 512], F32, tag="s")
                    for c in range(2):
                        nc.tensor.matmul(
                            s_t[:, c, :NC0],
                            lhsT=k_sb[b][:, hc, ts(mt, MT)],
                            rhs=q4[:, h, c * NC0:(c + 1) * NC0],
                            start=True, stop=False,
                            skip_group_check=True)
                    for c in range(2):
                        nc.tensor.matmul(
                            s_t[:, c, :NC0],
                            lhsT=ident[:],
                            rhs=reps[j][:, 14 * c:14 * c + 14, :W],
                            start=False, stop=True,
                            skip_group_check=True)
                    pt = ptpool.tile([MT, N], BF16, tag="pt")
                    nc.scalar.activation(
                        pt[:].rearrange("p (c n) -> p c n", c=2),
                        s_t[:, :, :NC0], AFT.Exp)
                    pend.append((b, t, j, mt, pt))
                    if len(pend) > AVLAG:
                        emit_av(*pend.pop(0))
            while pend:
                emit_av(*pend.pop(0))

        # ------------------------------------------- phase 3: out-projection
        with tc.tile_pool(name="pp3", bufs=2, space="PSUM") as pp3, \
             tc.tile_pool(name="osb", bufs=2) as osb:
            for b in range(BPC):
                for oc in range(3):
                    ps = pp3.tile([128, 2, 512], F32, tag="po")
                    for kc in range(6):
                        for c, (n0, nn) in enumerate(NCHUNKS):
                            nc.tensor.matmul(
                                ps[:, c, :nn],
                                lhsT=wpT[b][:, kc, ts(oc, 128)],
                                rhs=omid[b][:, kc, n0:n0 + nn],
                                start=(kc == 0), stop=(kc == 5))
                    o_t = osb.tile([128, N], F32, tag="ot")
                    for c, (n0, nn) in enumerate(NCHUNKS):
                        nc.vector.tensor_scalar(
                            o_t[:, n0:n0 + nn], ps[:, c, :nn],
                            bp_sb[:, oc:oc + 1], gm_sb[:, oc:oc + 1],
                            AOP.add, AOP.mult)
                    nc.sync.dma_start(out_d[b, ts(oc, 128), :], o_t[:])

    nc.compile()
    return nc


_NC_CACHE = None


def _get_program():
    global _NC_CACHE
    if _NC_CACHE is None:
        _NC_CACHE = _build_program()
    return _NC_CACHE


def _host_prep(inputs):
    """Shard/layout prep (pure slicing / transposition, no math)."""
    x = np.asarray(inputs["x"], np.float32).reshape(B, C_IN, N)
    Wq = np.asarray(inputs["Wq"], np.float32)
    Wkv = np.asarray(inputs["Wkv"], np.float32)
    Wproj = np.asarray(inputs["Wproj"], np.float32)
    bq = np.asarray(inputs["bq"], np.float32)
    bkv = np.asarray(inputs["bkv"], np.float32)
    bproj = np.asarray(inputs["bproj"], np.float32)
    gamma = np.asarray(inputs["gamma"], np.float32)
    bt = np.asarray(inputs["bias_table"], np.float32)

    wqT = np.ascontiguousarray(Wq.T)
    wkT = np.ascontiguousarray(Wkv[:MID].T)
    wvT = np.ascontiguousarray(Wkv[MID:].T)
    WT = np.ascontiguousarray(Wproj.T)          # [mid, out]
    wpT0 = np.zeros((768, OUT), np.float32)     # b0: rows 0-31 / 64-95 per tile
    wpT1 = np.zeros((768, OUT), np.float32)     # b1: rows 32-63 / 96-127
    for t in range(6):
        wpT0[128 * t:128 * t + 32] = WT[64 * t:64 * t + 32]
        wpT0[128 * t + 64:128 * t + 96] = WT[64 * t + 32:64 * t + 64]
        wpT1[128 * t + 32:128 * t + 64] = WT[64 * t:64 * t + 32]
        wpT1[128 * t + 96:128 * t + 128] = WT[64 * t + 32:64 * t + 64]
    db = np.ascontiguousarray(bt.T)             # [heads, 3025]

    shared = {
        "wqT": wqT, "wkT": wkT, "wvT": wvT, "wpT0": wpT0, "wpT1": wpT1,
        "bq": bq, "bk": bkv[:MID],
        "bp": bproj + Wproj @ bkv[MID:], "gm": gamma, "db": db,
    }
    in_maps = []
    for c in range(NCORES):
        m = dict(shared)
        m["x"] = np.ascontiguousarray(x[BPC * c:BPC * (c + 1)])
        in_maps.append(m)
    return in_maps


def kernel(**inputs) -> np.ndarray:
    from concourse.bass_utils import run_bass_kernel_spmd

    nc = _get_program()
    in_maps = _host_prep(inputs)
    res = run_bass_kernel_spmd(nc, in_maps, core_ids=list(range(NCORES)))
    outs = [res.results[c]["out"] for c in range(NCORES)]
    full = np.concatenate(outs, axis=0)          # [16, 384, 784]
    return np.ascontiguousarray(full.reshape(B, OUT, H, W).astype(np.float32))


if __name__ == "__main__":
    prog = _get_program()
    print("program built ok")
